# revision 15
# baseline (speedup 1.0000x reference)
"""CoralLoss (ordinal BCE-with-logits, mean reduction) on 8 Trainium2 cores.

Math: loss = mean over (B, K) of  max(x,0) - x*level + log1p(exp(-|x|))
with level[i,k] = (targets[i] > k).  Per element this equals
softplus(x) - x*level.  Using softplus(x) = x - ln(sigmoid(x)):

    sum(loss) = sum((1 - m) * x) - sum(ln(sigmoid(x))),  m[i,k] = (k < t_i)

Layout trick: the host sorts rows by target (the mean is permutation
invariant), so each 64-row partition-line shares a single target value.
At the <=101 t-boundary lines the line's first (minimum) t is used for
all 64 rows; the dropped terms are a zero-mean sum of ~6K independent
N(0,1) logits (~1e-5 relative on the mean, vs 2e-2 tolerance), since
logits and targets are independent.

Per-core plan (ROWS=32768 rows per core, 4 blocks of [128 partitions x
(64 rows * 100 cols)]):

 - ScalarE: ONE sigmoid pass s = sigmoid(x) (~21us/core floor; reads fp8
   logits, ACT is dtype-independent) + ONE small Ln over chunk products.
   sum(ln(s)) is regrouped as sum_chunks ln(prod_chunk s) with chunks of
   16: sigmoid products only shrink (typical chunk ~1e-5, worst realistic
   ~1e-30, bf16 min normal 1.2e-38), so no scaling or overflow handling
   is needed.  One activation-table switch (sigmoid set -> ln set)
   happens once, after the last sigmoid.
 - VectorE: only the 4-level pairwise product tree (tensor_tensor, 2x
   mode) + tiny masks - no elementwise add pass at all.
 - Mask: per line, mm[p, g*100+k] = (k >= t_p) is g-periodic, so a tiny
   extended tile mm_ext[p, j] = (j mod 100 >= t_p), j < 228, serves every
   [128,128] PE chunk c through the contiguous slice at (128*c) mod 100.
 - PE: sum(x*(1-m)) as 50 accumulating [128,128] matmuls per block into
   one PSUM tile (mask bf16 stationary, fp8 logits moving); trace(PSUM)
   is the dot.
 - Finale: diag-sum of the PSUM (prepared as soon as PE finishes), then
   total = diag_sum - ln_accum, partition-reduce via a ones-matmul; host
   sums 8 partials and divides by B*K.
 - Logits travel as fp8e4m3 (host-cast): halves HBM traffic vs bf16; the
   quantization bias on the mean is ~2e-4 relative.
"""

import numpy as np

import concourse.bacc as bacc
import concourse.tile as tile
from concourse import mybir
from concourse.bass_utils import run_bass_kernel_spmd

B = 262144
K = 100
M = 8                      # cores
ROWS = B // M              # 32768 rows per core
P = 128                    # SBUF partitions
G = 64                     # rows per partition-line per block
NBLK = ROWS // (P * G)     # 4 blocks per core
FB = G * K                 # 6400 free elements per partition per block
EXT = K + P                # extended mask width: mod-100 window for any chunk
NC128 = FB // P            # 50 PE chunks per block
PRW = FB // 16             # 400 chunk-products per block (chunks of 16)

_NC_CACHE = {}


def _tree(nc, w_ap, l1, l2, dst, width):
    """4 halving product levels: [P, width] -> dst [P, width/16]."""
    h, q, e, s = width // 2, width // 4, width // 8, width // 16
    nc.vector.tensor_tensor(
        out=l1[:, :h], in0=w_ap[:, :h], in1=w_ap[:, h:width],
        op=mybir.AluOpType.mult,
    )
    nc.vector.tensor_tensor(
        out=l2[:, :q], in0=l1[:, :q], in1=l1[:, q:h], op=mybir.AluOpType.mult
    )
    nc.vector.tensor_tensor(
        out=l1[:, :e], in0=l2[:, :e], in1=l2[:, e:q], op=mybir.AluOpType.mult
    )
    nc.vector.tensor_tensor(
        out=dst, in0=l1[:, :s], in1=l1[:, s:e], op=mybir.AluOpType.mult
    )


def _build_nc():
    nc = bacc.Bacc(None, target_bir_lowering=False)
    bf16 = mybir.dt.bfloat16
    fp8 = mybir.dt.float8e4
    f32 = mybir.dt.float32

    x_d = nc.dram_tensor("logits", [ROWS, K], fp8, kind="ExternalInput")
    tl_d = nc.dram_tensor("tlines", [P, NBLK], f32, kind="ExternalInput")
    ie_d = nc.dram_tensor("biota_ext", [P, EXT], bf16, kind="ExternalInput")
    id_d = nc.dram_tensor("ident", [P, P], bf16, kind="ExternalInput")
    out_d = nc.dram_tensor("partial", [1, 1], f32, kind="ExternalOutput")

    # block b, partition p holds rows [b*P*G + p*G, b*P*G + (p+1)*G) contiguous
    xv = x_d.rearrange("(b p g) k -> b p (g k)", p=P, g=G)

    with tile.TileContext(nc) as tc:
        with (
            tc.tile_pool(name="xblk", bufs=3) as xpool,
            tc.tile_pool(name="u", bufs=2) as upool,
            tc.tile_pool(name="l1", bufs=2) as l1pool,
            tc.tile_pool(name="l2", bufs=2) as l2pool,
            tc.tile_pool(name="dump", bufs=1) as dpool,
            tc.tile_pool(name="singles", bufs=1) as spool,
            tc.tile_pool(name="psum", bufs=1, space="PSUM") as ppool,
        ):
            x_tiles = {}

            # x block 0 in quarter/quarter/half pieces first (Act starts on
            # the first quarter), then x1, then consts, then x 2..3 whole
            xb0 = xpool.tile([P, FB], fp8)
            for lo, hi in ((0, FB // 4), (FB // 4, FB // 2), (FB // 2, FB)):
                nc.sync.dma_start(out=xb0[:, lo:hi], in_=xv[0][:, lo:hi])
            x_tiles[0] = xb0

            xb1 = xpool.tile([P, FB], fp8)
            nc.sync.dma_start(out=xb1, in_=xv[1][:, :])
            x_tiles[1] = xb1

            iext_t = spool.tile([P, EXT], bf16)
            nc.sync.dma_start(out=iext_t, in_=ie_d[:, :])
            tlines_t = spool.tile([P, NBLK], f32)
            nc.sync.dma_start(out=tlines_t, in_=tl_d[:, :])
            ident_t = spool.tile([P, P], bf16)
            nc.sync.dma_start(out=ident_t, in_=id_d[:, :])

            for b in range(2, NBLK):
                xb = xpool.tile([P, FB], fp8)
                nc.sync.dma_start(out=xb, in_=xv[b][:, :])
                x_tiles[b] = xb

            sp_row = spool.tile([P, 1], f32)
            pr_all = spool.tile([P, NBLK * PRW], bf16)
            psum_xl = ppool.tile([P, P], f32)

            # inverted masks (k >= t): tiny [P, EXT] tiles, one 4x ts each
            m_tiles = {}
            for b in range(NBLK):
                mx = spool.tile([P, EXT], bf16, name=f"mext{b}")
                nc.vector.tensor_scalar(
                    out=mx, in0=iext_t, scalar1=tlines_t[:, b : b + 1],
                    scalar2=None, op0=mybir.AluOpType.is_ge,
                )
                m_tiles[b] = mx

            # PE: psum += mm^T @ x in [128,128] chunks; lhsT slides through
            # the extended mask at offset (128*c) mod 100
            for b in range(NBLK):
                mx, xb = m_tiles[b], x_tiles[b]
                for c in range(NC128):
                    off = (c * P) % K
                    nc.tensor.matmul(
                        out=psum_xl,
                        lhsT=mx[:, off : off + P],
                        rhs=xb[:, c * P : (c + 1) * P],
                        start=(b == 0 and c == 0),
                        stop=(b == NBLK - 1 and c == NC128 - 1),
                    )

            # trace(psum) pieces, ready as soon as PE finishes
            diag = spool.tile([P, P], f32)
            xl_row = spool.tile([P, 1], f32)

            # Act sigmoid + DVE product tree, per block.  Block 0's sigmoid
            # runs quarter/quarter/half (early start); block 3 runs in halves
            # with quarter-width trees so the final chain finishes during the
            # activation-table switch; middle blocks run whole.
            for b in range(NBLK):
                xb = x_tiles[b]
                ub = upool.tile([P, FB], bf16)
                l1 = l1pool.tile([P, FB // 2], bf16)
                l2 = l2pool.tile([P, FB // 2], bf16)
                if b == 0:
                    act_parts = ((0, FB // 4), (FB // 4, FB // 2), (FB // 2, FB))
                elif b == 1:
                    act_parts = ((0, FB),)
                else:
                    act_parts = ((0, FB // 2), (FB // 2, FB))
                for lo, hi in act_parts:
                    nc.scalar.activation(
                        out=ub[:, lo:hi], in_=xb[:, lo:hi],
                        func=mybir.ActivationFunctionType.Sigmoid,
                    )
                    _tree(
                        nc, ub[:, lo:hi], l1, l2,
                        pr_all[:, b * PRW + lo // 16 : b * PRW + hi // 16],
                        hi - lo,
                    )


            # trace(psum): emitted after all trees so it can't head-of-line
            # block the Vector queue behind the PE's last matmul
            nc.vector.tensor_mul(diag, psum_xl[:, :], ident_t[:, :])
            nc.vector.reduce_sum(out=xl_row, in_=diag, axis=mybir.AxisListType.X)

            # chunk-Ln (one table switch), split so blocks 0-2 get their Ln
            # right after the switch and block 3's small Ln lands as soon as
            # its last tree finishes
            sp_row2 = spool.tile([P, 1], f32)
            lnd = dpool.tile([P, NBLK * PRW], bf16)
            nc.scalar.activation(
                out=lnd[:, : 3 * PRW], in_=pr_all[:, : 3 * PRW],
                func=mybir.ActivationFunctionType.Ln,
                accum_out=sp_row,
            )
            nc.scalar.activation(
                out=lnd[:, 3 * PRW :], in_=pr_all[:, 3 * PRW :],
                func=mybir.ActivationFunctionType.Ln,
                accum_out=sp_row2,
            )

            # total = sum((1-m) x) - sum(ln sigmoid)
            tot = spool.tile([P, 1], f32)
            nc.vector.tensor_tensor(
                out=tot, in0=xl_row, in1=sp_row, op=mybir.AluOpType.subtract
            )
            nc.vector.tensor_tensor(
                out=tot, in0=tot, in1=sp_row2, op=mybir.AluOpType.subtract
            )

            ones_t = spool.tile([P, 1], f32)
            nc.vector.memset(ones_t, 1.0)
            psum_tot = ppool.tile([1, 1], f32)
            nc.tensor.matmul(
                out=psum_tot, lhsT=tot, rhs=ones_t, start=True, stop=True
            )
            res = spool.tile([1, 1], f32)
            nc.vector.tensor_copy(res, psum_tot)
            nc.sync.dma_start(out=out_d[:, :], in_=res)
    nc.finalize()
    return nc


def _run(logits, targets, trace=False, trace_kwargs=None):
    import ml_dtypes

    logits = np.ascontiguousarray(np.asarray(logits), dtype=np.float32)
    targets = np.asarray(targets)
    assert logits.shape == (B, K), logits.shape
    assert targets.shape == (B,), targets.shape

    if "nc" not in _NC_CACHE:
        _NC_CACHE["nc"] = _build_nc()
    nc = _NC_CACHE["nc"]

    bf16 = ml_dtypes.bfloat16
    fp8 = ml_dtypes.float8_e4m3fn

    # sort rows by target so each 64-row partition-line is t-homogeneous
    order = np.argsort(targets, kind="stable")
    logits8 = logits[order].astype(fp8)
    t_sorted = targets[order].astype(np.float32)

    iext = np.ascontiguousarray(
        np.broadcast_to(
            np.arange(EXT, dtype=np.float32) % K, (P, EXT)
        )
    ).astype(bf16)
    ident = np.eye(P, dtype=np.float32).astype(bf16)

    in_maps = []
    for c in range(M):
        xs = logits8[c * ROWS : (c + 1) * ROWS]
        ts = t_sorted[c * ROWS : (c + 1) * ROWS]
        # t of the first row of each (block, partition) line
        tlines = ts.reshape(NBLK, P, G)[:, :, 0].transpose(1, 0)  # [P, NBLK]
        in_maps.append(
            {
                "logits": xs,
                "tlines": np.ascontiguousarray(tlines, dtype=np.float32),
                "biota_ext": iext,
                "ident": ident,
            }
        )

    res = run_bass_kernel_spmd(
        nc, in_maps, core_ids=list(range(M)), trace=trace, **(trace_kwargs or {})
    )
    total = sum(float(res.results[c]["partial"][0, 0]) for c in range(M))
    out = np.array(total / (B * K), dtype=np.float32)
    return out, res


def kernel(logits, targets):
    out, _ = _run(logits, targets)
    return out


# revision 19
# speedup vs baseline: 1.0806x; 1.0806x over previous
"""CoralLoss (ordinal BCE-with-logits, mean reduction) on 8 Trainium2 cores.

Math: loss = mean over (B, K) of  max(x,0) - x*level + log1p(exp(-|x|))
with level[i,k] = (targets[i] > k).  Per element this equals
softplus(x) - x*level.  Using softplus(x) = x - ln(sigmoid(x)):

    sum(loss) = sum((1 - m) * x) - sum(ln(sigmoid(x))),  m[i,k] = (k < t_i)

Layout trick: the host sorts rows by target (the mean is permutation
invariant), so each 64-row partition-line shares a single target value.
At the <=101 t-boundary lines the line's first (minimum) t is used for
all 64 rows; the dropped terms are a zero-mean sum of ~6K independent
N(0,1) logits (~1e-5 relative on the mean, vs 2e-2 tolerance), since
logits and targets are independent.

Per-core plan (ROWS=32768 rows per core, 4 blocks of [128 partitions x
(64 rows * 100 cols)]):

 - ScalarE: ONE sigmoid pass s = sigmoid(x) (~21us/core floor; reads fp8
   logits, ACT is dtype-independent) + ONE small Ln over chunk products.
   sum(ln(s)) is regrouped as sum_chunks ln(prod_chunk s) with chunks of
   16: sigmoid products only shrink (typical chunk ~1e-5, worst realistic
   ~1e-30, bf16 min normal 1.2e-38), so no scaling or overflow handling
   is needed.  One activation-table switch (sigmoid set -> ln set)
   happens once, after the last sigmoid.
 - VectorE: only the 4-level pairwise product tree (tensor_tensor, 2x
   mode) + tiny masks - no elementwise add pass at all.
 - Mask: per line, mm[p, g*100+k] = (k >= t_p) is g-periodic, so a tiny
   extended tile mm_ext[p, j] = (j mod 100 >= t_p), j < 228, serves every
   [128,128] PE chunk c through the contiguous slice at (128*c) mod 100.
 - PE: sum(x*(1-m)) as 50 accumulating [128,128] matmuls per block into
   one PSUM tile (mask bf16 stationary, fp8 logits moving); trace(PSUM)
   is the dot.
 - Finale: diag-sum of the PSUM (prepared as soon as PE finishes), then
   total = diag_sum - ln_accum, partition-reduce via a ones-matmul; host
   sums 8 partials and divides by B*K.
 - Logits travel as fp8e4m3 (host-cast): halves HBM traffic vs bf16; the
   quantization bias on the mean is ~2e-4 relative.
"""

import numpy as np

import concourse.bacc as bacc
import concourse.tile as tile
from concourse import mybir
from concourse.bass_utils import run_bass_kernel_spmd

B = 262144
K = 100
M = 8                      # cores
ROWS = B // M              # 32768 rows per core
P = 128                    # SBUF partitions
G = 64                     # rows per partition-line per block
NBLK = ROWS // (P * G)     # 4 blocks per core
FB = G * K                 # 6400 free elements per partition per block
EXT = K + P                # extended mask width: mod-100 window for any chunk
NC128 = FB // P            # 50 PE chunks per block
PRW = FB // 16             # 400 chunk-products per block (chunks of 16)

_NC_CACHE = {}


def _tree(nc, w_ap, l1, l2, dst, width):
    """4 halving product levels: [P, width] -> dst [P, width/16]."""
    h, q, e, s = width // 2, width // 4, width // 8, width // 16
    nc.vector.tensor_tensor(
        out=l1[:, :h], in0=w_ap[:, :h], in1=w_ap[:, h:width],
        op=mybir.AluOpType.mult,
    )
    nc.vector.tensor_tensor(
        out=l2[:, :q], in0=l1[:, :q], in1=l1[:, q:h], op=mybir.AluOpType.mult
    )
    nc.vector.tensor_tensor(
        out=l1[:, :e], in0=l2[:, :e], in1=l2[:, e:q], op=mybir.AluOpType.mult
    )
    nc.vector.tensor_tensor(
        out=dst, in0=l1[:, :s], in1=l1[:, s:e], op=mybir.AluOpType.mult
    )


def _build_nc():
    nc = bacc.Bacc(None, target_bir_lowering=False)
    bf16 = mybir.dt.bfloat16
    fp8 = mybir.dt.float8e4
    f32 = mybir.dt.float32

    x_d = nc.dram_tensor("logits", [ROWS, K], fp8, kind="ExternalInput")
    tl_d = nc.dram_tensor("tlines", [P, NBLK], f32, kind="ExternalInput")
    ie_d = nc.dram_tensor("biota_ext", [P, EXT], bf16, kind="ExternalInput")
    id_d = nc.dram_tensor("ident", [P, P], bf16, kind="ExternalInput")
    out_d = nc.dram_tensor("partial", [1, 1], f32, kind="ExternalOutput")

    # block b, partition p holds rows [b*P*G + p*G, b*P*G + (p+1)*G) contiguous
    xv = x_d.rearrange("(b p g) k -> b p (g k)", p=P, g=G)

    with tile.TileContext(nc) as tc:
        with (
            tc.tile_pool(name="xblk", bufs=3) as xpool,
            tc.tile_pool(name="u", bufs=2) as upool,
            tc.tile_pool(name="l1", bufs=2) as l1pool,
            tc.tile_pool(name="l2", bufs=2) as l2pool,
            tc.tile_pool(name="dump", bufs=1) as dpool,
            tc.tile_pool(name="singles", bufs=1) as spool,
            tc.tile_pool(name="psum", bufs=1, space="PSUM") as ppool,
        ):
            x_tiles = {}

            # x block 0 in quarter/quarter/half pieces first (Act starts on
            # the first quarter), then x1, then consts, then x 2..3 whole
            xb0 = xpool.tile([P, FB], fp8)
            for lo, hi in ((0, FB // 4), (FB // 4, FB // 2), (FB // 2, FB)):
                nc.sync.dma_start(out=xb0[:, lo:hi], in_=xv[0][:, lo:hi])
            x_tiles[0] = xb0

            xb1 = xpool.tile([P, FB], fp8)
            nc.sync.dma_start(out=xb1, in_=xv[1][:, :])
            x_tiles[1] = xb1

            iext_t = spool.tile([P, EXT], bf16)
            nc.sync.dma_start(out=iext_t, in_=ie_d[:, :])
            tlines_t = spool.tile([P, NBLK], f32)
            nc.sync.dma_start(out=tlines_t, in_=tl_d[:, :])
            ident_t = spool.tile([P, P], bf16)
            nc.sync.dma_start(out=ident_t, in_=id_d[:, :])

            for b in range(2, NBLK):
                xb = xpool.tile([P, FB], fp8)
                nc.sync.dma_start(out=xb, in_=xv[b][:, :])
                x_tiles[b] = xb

            sp_row = spool.tile([P, 1], f32)
            pr_all = spool.tile([P, NBLK * PRW], bf16)
            psum_xl = ppool.tile([P, P], f32)

            # inverted masks (k >= t): tiny [P, EXT] tiles, one 4x ts each
            m_tiles = {}
            for b in range(NBLK):
                mx = spool.tile([P, EXT], bf16, name=f"mext{b}")
                nc.vector.tensor_scalar(
                    out=mx, in0=iext_t, scalar1=tlines_t[:, b : b + 1],
                    scalar2=None, op0=mybir.AluOpType.is_ge,
                )
                m_tiles[b] = mx

            # PE: psum += mm^T @ x in [128,128] chunks; lhsT slides through
            # the extended mask at offset (128*c) mod 100
            for b in range(NBLK):
                mx, xb = m_tiles[b], x_tiles[b]
                for c in range(NC128):
                    off = (c * P) % K
                    nc.tensor.matmul(
                        out=psum_xl,
                        lhsT=mx[:, off : off + P],
                        rhs=xb[:, c * P : (c + 1) * P],
                        start=(b == 0 and c == 0),
                        stop=(b == NBLK - 1 and c == NC128 - 1),
                    )

            # trace(psum) pieces, ready as soon as PE finishes
            diag = spool.tile([P, P], f32)
            xl_row = spool.tile([P, 1], f32)

            # Act sigmoid + DVE product tree, per block.  Block 0's sigmoid
            # runs quarter/quarter/half (early start); block 3 runs in halves
            # with quarter-width trees so the final chain finishes during the
            # activation-table switch; middle blocks run whole.
            for b in range(NBLK):
                xb = x_tiles[b]
                ub = upool.tile([P, FB], bf16)
                l1 = l1pool.tile([P, FB // 2], bf16)
                l2 = l2pool.tile([P, FB // 2], bf16)
                if b == 0:
                    act_parts = ((0, FB // 4), (FB // 4, FB // 2), (FB // 2, FB))
                elif b == 1:
                    act_parts = ((0, FB),)
                else:
                    act_parts = ((0, FB // 2), (FB // 2, FB))
                for lo, hi in act_parts:
                    nc.scalar.activation(
                        out=ub[:, lo:hi], in_=xb[:, lo:hi],
                        func=mybir.ActivationFunctionType.Sigmoid,
                    )
                    _tree(
                        nc, ub[:, lo:hi], l1, l2,
                        pr_all[:, b * PRW + lo // 16 : b * PRW + hi // 16],
                        hi - lo,
                    )


            # trace(psum).  The scheduler likes to hoist this ahead of the
            # later trees, head-of-line blocking the Vector queue behind the
            # PE's last matmul; forcing a data dependency on the final tree
            # output (bypass ts whose ignored scalar reads pr_all's last
            # column) pins it after all tree work.
            ident_c = spool.tile([P, P], bf16)
            nc.vector.tensor_scalar(
                out=ident_c, in0=ident_t,
                scalar1=pr_all[:, NBLK * PRW - 2 : NBLK * PRW].bitcast(f32),
                scalar2=None, op0=mybir.AluOpType.bypass,
            )
            nc.vector.tensor_mul(diag, psum_xl[:, :], ident_c[:, :])
            nc.vector.reduce_sum(out=xl_row, in_=diag, axis=mybir.AxisListType.X)

            # chunk-Ln (one table switch), split so blocks 0-2 get their Ln
            # right after the switch and block 3's small Ln lands as soon as
            # its last tree finishes
            sp_row2 = spool.tile([P, 1], f32)
            lnd = dpool.tile([P, NBLK * PRW], bf16)
            nc.scalar.activation(
                out=lnd[:, : 3 * PRW], in_=pr_all[:, : 3 * PRW],
                func=mybir.ActivationFunctionType.Ln,
                accum_out=sp_row,
            )
            nc.scalar.activation(
                out=lnd[:, 3 * PRW :], in_=pr_all[:, 3 * PRW :],
                func=mybir.ActivationFunctionType.Ln,
                accum_out=sp_row2,
            )

            # total = sum((1-m) x) - sum(ln sigmoid), one fused subtract
            tot = spool.tile([P, 1], f32)
            nc.vector.scalar_tensor_tensor(
                out=tot, in0=xl_row, scalar=sp_row[:, 0:1], in1=sp_row2,
                op0=mybir.AluOpType.subtract, op1=mybir.AluOpType.subtract,
            )

            ones_t = spool.tile([P, 1], f32)
            nc.vector.memset(ones_t, 1.0)
            psum_tot = ppool.tile([1, 1], f32)
            nc.tensor.matmul(
                out=psum_tot, lhsT=tot, rhs=ones_t, start=True, stop=True
            )
            res = spool.tile([1, 1], f32)
            nc.scalar.copy(res, psum_tot)
            nc.sync.dma_start(out=out_d[:, :], in_=res)
    nc.finalize()
    return nc


def _run(logits, targets, trace=False, trace_kwargs=None):
    import ml_dtypes

    logits = np.ascontiguousarray(np.asarray(logits), dtype=np.float32)
    targets = np.asarray(targets)
    assert logits.shape == (B, K), logits.shape
    assert targets.shape == (B,), targets.shape

    if "nc" not in _NC_CACHE:
        _NC_CACHE["nc"] = _build_nc()
    nc = _NC_CACHE["nc"]

    bf16 = ml_dtypes.bfloat16
    fp8 = ml_dtypes.float8_e4m3fn

    # sort rows by target so each 64-row partition-line is t-homogeneous
    order = np.argsort(targets, kind="stable")
    logits8 = logits[order].astype(fp8)
    t_sorted = targets[order].astype(np.float32)

    iext = np.ascontiguousarray(
        np.broadcast_to(
            np.arange(EXT, dtype=np.float32) % K, (P, EXT)
        )
    ).astype(bf16)
    ident = np.eye(P, dtype=np.float32).astype(bf16)

    in_maps = []
    for c in range(M):
        xs = logits8[c * ROWS : (c + 1) * ROWS]
        ts = t_sorted[c * ROWS : (c + 1) * ROWS]
        # t of the first row of each (block, partition) line
        tlines = ts.reshape(NBLK, P, G)[:, :, 0].transpose(1, 0)  # [P, NBLK]
        in_maps.append(
            {
                "logits": xs,
                "tlines": np.ascontiguousarray(tlines, dtype=np.float32),
                "biota_ext": iext,
                "ident": ident,
            }
        )

    res = run_bass_kernel_spmd(
        nc, in_maps, core_ids=list(range(M)), trace=trace, **(trace_kwargs or {})
    )
    total = sum(float(res.results[c]["partial"][0, 0]) for c in range(M))
    out = np.array(total / (B * K), dtype=np.float32)
    return out, res


def kernel(logits, targets):
    out, _ = _run(logits, targets)
    return out
